# revision 19
# baseline (speedup 1.0000x reference)
"""Trainium2 Bass kernel for nn_CausalAttention_76304388981436.

Full-input contract: kernel(**inputs) -> [2, 2048, 512] f32.

Sharding (8 cores, single SPMD program): core c = (batch b=c//4, head-pair
hp=c%4).  Each core computes attention for its 2 heads over the full 2048
sequence of its batch, producing per-head UNNORMALIZED projected numerators
outh[h] = (sum_j exp(S-11) vT)^T @ Wo[h-rows]  [2048, 512] f16 plus the
softmax denominators dens2 [4, 2, 512] f16; the host divides per head, sums
the 4 head-pair partials per batch and adds bo.

Device-side math per core (transposed-attention layout, heads packed at
partitions 0-63 / 64-127 of the PE array):
  qT[128i, 2048n] = wq2^T x^T   (wq2 pre-scaled by 1/8 on host; all inputs
                                 bf16 on host to halve DMA + run PE 1cyc/row)
  kT[128i, 2048n] = wk2^T x^T
  vta[128n, 16t, 2h, 65] = x wv2 (+ ones col 64 -> PV also produces denom)
  P' tiles [128q, 512s] = qT_h^T rel_embT_rev  (rel table pre-reversed on
      host; only the live s-chunks per q-chunk are computed/stored)
  PR dram per qc [2h, 512q, 2048s] fp16: bias[j,q] = PR[h, q, j-q+1023]
  skew read: ONE transposing DMA per (h, qc) with source AP
      [[2047, 512], [1, 128*njt]] (contiguous 2-3KB runs) -> SBUF
      [128j, njt, 512q]; h0 on the SP HWDGE queue, h1 on the ACT HWDGE
      queue so the two transfers overlap.
  S^T tile [128j, 1024(h0q|h1q)] = kT_h^T qT_h (both heads tile_position-
      paired); bias added via accumulating ident_f16 matmul (in-band) or
      clamp-value matmul (out-of-band) into the same PSUM group.
  expS = exp(S^T - 11)  one ACT op per [128, 1024] jt tile
  outT_h [65, 512q] += vta_h^T expS   (row 64 = denominator)
  outh[h] [2048q, 512] = numT_h^T wo2_h  (unnormalized, fp16 out)

Scheduling: per q-chunk the OUT-OF-BAND j-tiles run first so the skew DMA
(issued mid-previous-iteration right after the PR writes) overlaps compute.
P'(qc+1) units are front-loaded as PE filler inside attn(qc); outproj(qc-1)
trails at the iteration end.  Prologue pipelines x-chunk DMAs with q-proj,
P'(0), k/v-proj so the PE never sits idle waiting for input DMAs.
"""
import numpy as np
import ml_dtypes

import concourse.bass as bass
import concourse.mybir as mybir
import concourse.tile as tile
from concourse.bass_utils import run_bass_kernel_spmd

F32 = mybir.dt.float32
F32R = mybir.dt.float32r
F16 = mybir.dt.float16
BF16 = mybir.dt.bfloat16
AF = mybir.ActivationFunctionType

N = 2048          # sequence length
D = 512           # model dim
HD = 64           # head dim
NQC = 4           # q-chunks of 512
NJT = 16          # j-tiles of 128
W = 2048          # PR row width
SHIFT = -11.0     # exp(logit + SHIFT): keeps num/den in fp16 range

# per q-chunk: (jt_min, njt) of in-band j-tiles (-512 < A < 1152,
# A = 512*(qc+1) - 128*jt)
IN_BAND = {0: (0, 8), 1: (0, 12), 2: (4, 12), 3: (8, 8)}
# per q-chunk: live 512-wide s-chunks of PR (others never read)
LIVE_CI = {0: (1, 2, 3), 1: (0, 1, 2, 3), 2: (0, 1, 2, 3), 3: (0, 1, 2)}


def _split_multiwaits(nc):
    """This walrus build rejects >1 sync wait per instruction; split extra
    waits onto single-wait NoOps on the same engine just before it."""
    for func in nc.m.functions:
        for block in func.blocks:
            new_instrs = []
            for inst in block.instructions:
                si = inst.sync_info
                if si is not None and si.on_wait and len(si.on_wait) > 1:
                    waits = list(si.on_wait)
                    for w in waits[:-1]:
                        new_instrs.append(mybir.InstNoOp(
                            name=nc.get_next_instruction_name(),
                            engine=inst.engine,
                            bass_nofuse=True,
                            sync_info=mybir.SyncInfo(on_wait=[w], on_update=[]),
                        ))
                    si.on_wait = waits[-1:]
                new_instrs.append(inst)
            block.instructions = new_instrs


def _r(ap):
    return ap.bitcast(F32R)


def build_kernel():
    nc = bass.Bass("TRN2")
    xT = nc.dram_tensor("xT", [D, N], BF16, kind="ExternalInput")
    wqkv = nc.dram_tensor("wqkv", [4, 128, 3, 128], BF16, kind="ExternalInput")
    wo2 = nc.dram_tensor("wo2", [128, D], F32, kind="ExternalInput")
    relT = nc.dram_tensor("relT", [128, W], BF16, kind="ExternalInput")
    relbc = nc.dram_tensor("relbc", [128, 256], BF16, kind="ExternalInput")
    ident = nc.dram_tensor("ident", [128, 128], F16, kind="ExternalInput")
    outh = nc.dram_tensor("outh", [2, N, D], F16, kind="ExternalOutput")
    dens2 = nc.dram_tensor("dens2", [4, 2, 512], F16, kind="ExternalOutput")

    with tile.TileContext(nc) as tc:
        _build_body(nc, tc, xT, wqkv, wo2, relT, relbc, ident, outh, dens2)
    _split_multiwaits(nc)
    return nc


def _build_body(nc, tc, xT, wqkv, wo2, relT, relbc, ident, outh, dens2):
    from contextlib import ExitStack
    ctx = ExitStack()
    consts = ctx.enter_context(tc.tile_pool(name="consts", bufs=1))
    qkv = ctx.enter_context(tc.tile_pool(name="qkv", bufs=1))
    pc = ctx.enter_context(tc.tile_pool(name="pc", bufs=2))
    skew = ctx.enter_context(tc.tile_pool(name="skew", bufs=2))
    exps = ctx.enter_context(tc.tile_pool(name="exps", bufs=6))
    outc = ctx.enter_context(tc.tile_pool(name="outc", bufs=4))
    dent = ctx.enter_context(tc.tile_pool(name="dent", bufs=2))
    ring = ctx.enter_context(tc.tile_pool(name="ring", bufs=3, space="PSUM"))
    psum = ctx.enter_context(tc.tile_pool(name="psum", bufs=1, space="PSUM"))
    pdram = ctx.enter_context(tc.tile_pool(name="pdram", bufs=1, space="DRAM"))

    # ---- input DMAs (constants on the Pool SWDGE queue, x on SP/ACT) ----
    # order by first use: wqkv (q proj) -> relT (P'0) -> relbc/ident (attn0)
    # -> wo (outproj, attn1)
    swqkv = consts.tile([128, 4, 3, 128], BF16, name="wqkv", tag="wqkv")
    nc.gpsimd.dma_start(
        out=swqkv[:],
        in_=wqkv.rearrange("c p k m -> p c k m"))
    srelT = consts.tile([128, W], BF16, name="relT", tag="relT")
    nc.gpsimd.dma_start(out=srelT[:], in_=relT[:, :])
    srelbc = consts.tile([128, 256], BF16, name="relbc", tag="relbc")
    nc.gpsimd.dma_start(out=srelbc[:], in_=relbc[:, :])
    sident = consts.tile([128, 128], F16, name="ident", tag="ident")
    nc.gpsimd.dma_start(out=sident[:], in_=ident[:, :])
    swo = consts.tile([128, D], F32, name="wo", tag="wo")
    nc.gpsimd.dma_start(out=_r(swo[:]), in_=_r(wo2[:, :]))
    sbias = consts.tile([128, 1], F32, name="sbias", tag="sbias")
    nc.gpsimd.memset(sbias[:], SHIFT)

    sxT = [consts.tile([128, N], BF16, name=f"xT{i}", tag=f"xT{i}")
           for i in range(4)]
    for half in range(2):
        ns = slice(half * 1024, half * 1024 + 1024)
        for i in range(4):
            eng = nc.sync if i % 2 == 0 else nc.scalar
            eng.dma_start(out=sxT[i][:, ns], in_=xT[i * 128:(i + 1) * 128, ns])

    # ---- SBUF working tensors ----
    qT = qkv.tile([128, N], BF16, name="qT", tag="qT")
    kT = qkv.tile([128, N], BF16, name="kT", tag="kT")
    # v with ones column per (t, head): [n-in-tile, t, h, d|1]
    vta = qkv.tile([128, NJT, 2, 65], BF16, name="vta", tag="vta")
    nc.gpsimd.memset(vta[:, :, :, 64:65], 1.0)
    # ah holds both heads' unnormalized numerators: rows 0-63 h0, 64-127 h1
    ah = qkv.tile([128, N], F32, name="ah", tag="ah")

    # PSUM: one FIFO ring of 3 x [128,1024] (6 banks) shared by ALL
    # paired-matmul outputs (S tiles, P' units, proj, outproj) + 2 banks
    # for the PV accumulators = 8 banks.  The ring decouples each unit's
    # matmuls from its own consumer (exp/cast) by ~2 units of pipeline
    # distance and keeps h0/h1 tile_position pairing intact.
    def mm_tile():
        return ring.tile([128, 1024], F32, name="mm", tag="mm")

    # ---- per-qc PR dram tensors [2h, 512q, W] ----
    prd = {qc: pdram.tile([2, 512, W], F16, name=f"pr{qc}", tag=f"pr{qc}")
           for qc in range(NQC)}

    # ---- projection units ----
    def q_pair_unit(pair):
        ps = mm_tile()
        for nl in range(2):
            nch = 2 * pair + nl
            ns = slice(nch * 512, nch * 512 + 512)
            for c in range(4):
                nc.tensor.matmul(ps[:, nl * 512:nl * 512 + 512],
                                 swqkv[:, c, 0, :], sxT[c][:, ns],
                                 start=(c == 0), stop=(c == 3))
        nc.vector.tensor_copy(out=qT[:, pair * 1024:pair * 1024 + 1024],
                              in_=ps[:])

    def k_pair_unit(pair):
        ps = mm_tile()
        for nl in range(2):
            nch = 2 * pair + nl
            ns = slice(nch * 512, nch * 512 + 512)
            for c in range(4):
                nc.tensor.matmul(ps[:, nl * 512:nl * 512 + 512],
                                 swqkv[:, c, 1, :], sxT[c][:, ns],
                                 start=(c == 0), stop=(c == 3))
        nc.vector.tensor_copy(out=kT[:, pair * 1024:pair * 1024 + 1024],
                              in_=ps[:])

    def v_group_unit(g):
        # tiles t = 4g .. 4g+3 -> one [128, 512] half of a ring tile
        ps = mm_tile()
        half = ps[:, 0:512]
        for tl in range(4):
            t = 4 * g + tl
            nst = slice(t * 128, t * 128 + 128)
            for c in range(4):
                nc.tensor.matmul(half[:, tl * 128:tl * 128 + 128],
                                 sxT[c][:, nst], swqkv[:, c, 2, :],
                                 start=(c == 0), stop=(c == 3))
        nc.vector.tensor_copy(
            out=vta[:, 4 * g:4 * g + 4, :, 0:64],
            in_=half.rearrange("p (t h d) -> p t h d", t=4, h=2))

    # ---- P' production units for q-chunk qc ----
    def p_units_for(qc, act_casts=False):
        """Fine-grained closures producing P'(qc): one per (qt, ci) matmul+
        copy step, plus one per qt for the PR write DMA.  act_casts=True
        alternates the PSUM->SBUF casts between DVE and ACT — only safe in
        the prologue where the ACT engine has no exp stream to block."""
        if qc is None or qc >= NQC:
            return []
        units = []
        jt_min, njt = IN_BAND[qc]
        off = 128 * jt_min - 512 * qc + 1023
        state = {}
        nu = 0
        for qt_local in range(4):
            qt = 4 * qc + qt_local
            # exact band window for this qt's rows, 256-aligned: the skew
            # parallelogram reads cols [off-a, off-a+128*njt) for each row a
            amin, amax = 128 * qt_local, 128 * qt_local + 127
            lo = max(0, (off - amax) // 256 * 256)
            hi = min(W, -((off - amin + 128 * njt) // -256) * 256)

            w0s = list(range(lo, hi, 512))
            for k, w0 in enumerate(w0s):
                cw = min(512, hi - w0)
                nu += 1
                def do_ci(qt=qt, w0=w0, cw=cw, first=(k == 0),
                          on_act=(act_casts and nu % 2 == 0)):
                    if first:
                        state[qt] = pc.tile([128, 2, W], F16, name="pct",
                                            tag="pct")
                    qs = slice(qt * 128, qt * 128 + 128)
                    ps = mm_tile()
                    for h in range(2):
                        hs = slice(h * 64, h * 64 + 64)
                        nc.tensor.matmul(ps[:, h * 512:h * 512 + cw],
                                         qT[hs, qs], srelT[hs, w0:w0 + cw],
                                         start=True, stop=True,
                                         tile_position=(h * 64, 0))
                    hv = ps[:].rearrange("p (h s) -> p h s", h=2)[:, :, 0:cw]
                    if on_act:
                        nc.scalar.activation(out=state[qt][:, :, w0:w0 + cw],
                                             in_=hv, func=AF.Copy)
                    else:
                        nc.vector.tensor_copy(out=state[qt][:, :, w0:w0 + cw],
                                              in_=hv)

                units.append(do_ci)

            def write_qt(qt=qt, qt_local=qt_local, lo=lo, hi=hi):
                rows = slice(qt_local * 128, qt_local * 128 + 128)
                pct = state[qt]
                nc.gpsimd.dma_start(
                    out=prd[qc][:, rows, lo:hi].rearrange("h r w -> r h w"),
                    in_=pct[:, :, lo:hi])

            units.append(write_qt)
        return units

    def emit_skew_read(qc):
        """Transposing DMAs covering all in-band j-tiles.  All on the SP
        queue: concurrent XBAR transposes on different queues corrupt each
        other (verified on HW).  Each head is split in two halves ordered
        (h0a, h1a, h0b, h1b) so the first in-band tiles of both heads are
        available after ~half the total transfer time."""
        jt_min, njt = IN_BAND[qc]
        t = prd[qc]
        tiles = {}
        for h in range(2):
            tiles[h] = skew.tile([128, 12, 512], F16, name="skt",
                                 tag=f"skt{h}")
        for t0 in range(0, njt, 2):
            tn = min(2, njt - t0)
            for h in range(2):
                src = bass.AP(tensor=t.tensor,
                              offset=t.offset + h * 512 * W + 128 * t0
                              + 128 * jt_min - 512 * qc + 1023,
                              ap=[[2047, 512], [1, 128 * tn]])
                nc.sync.dma_start(out=tiles[h][:, t0:t0 + tn, :], in_=src,
                                  transpose=True)
        return tiles

    # ---- output projection units for q-chunk qc (unnormalized) ----
    def o_units_for(qc):
        if qc is None or qc < 0:
            return []
        units = []
        for qt_local in range(4):
            qt = 4 * qc + qt_local

            def do_o(qt=qt):
                qs = slice(qt * 128, qt * 128 + 128)
                ps = mm_tile()
                for h in range(2):
                    hs = slice(h * 64, h * 64 + 64)
                    nc.tensor.matmul(ps[:, h * 512:h * 512 + 512],
                                     _r(ah[hs, qs]), _r(swo[hs, :]),
                                     start=True, stop=True,
                                     tile_position=(h * 64, 0))
                ot = outc.tile([128, 1024], F16, name="oc", tag="oc")
                nc.scalar.activation(out=ot[:], in_=ps[:], func=AF.Copy)
                nc.scalar.dma_start(
                    out=outh[:, qs, :].rearrange("h q d -> q h d"),
                    in_=ot[:].rearrange("p (h d) -> p h d", h=2))

            units.append(do_o)
        return units

    # ---- attention for one q-chunk ----
    def emit_attn(qc, skt, o_fills, drain_gq):
        jt_min, njt = IN_BAND[qc]
        in_band = lambda jt: jt_min <= jt < jt_min + njt
        jts = [jt for jt in range(NJT) if not in_band(jt)] + \
              [jt for jt in range(NJT) if in_band(jt)]
        qs = slice(qc * 512, qc * 512 + 512)
        pot = [psum.tile([65, 512], F32, name="po", tag=f"po{h}")
               for h in range(2)]
        ets = {}
        fu = iter(o_fills)

        def emit_pv(pi):
            jt = jts[pi]
            for h in range(2):
                nc.tensor.matmul(pot[h][:], vta[:, jt, h, :],
                                 ets[jt][:, h * 512:h * 512 + 512],
                                 start=(pi == 0), stop=(pi == NJT - 1))

        for pi, jt in enumerate(jts):
            js = slice(jt * 128, jt * 128 + 128)
            ps = mm_tile()
            for h in range(2):
                hs = slice(h * 64, h * 64 + 64)
                nc.tensor.matmul(ps[:, h * 512:h * 512 + 512],
                                 kT[hs, js], qT[hs, qs],
                                 start=True, stop=False,
                                 tile_position=(h * 64, 0))
            if in_band(jt):
                for h in range(2):
                    nc.tensor.matmul(ps[:, h * 512:h * 512 + 512],
                                     sident[:], skt[h][:, jt - jt_min, :],
                                     start=False, stop=True)
            else:
                A = qc * 512 + 512 - 128 * jt
                bc = 0 if A <= -512 else 128
                for h in range(2):
                    hs = slice(h * 64, h * 64 + 64)
                    nc.tensor.matmul(ps[:, h * 512:h * 512 + 512],
                                     srelbc[hs, bc:bc + 128], qT[hs, qs],
                                     start=False, stop=True,
                                     tile_position=(h * 64, 0))
            et = exps.tile([128, 1024], BF16, name="expS", tag="expS")
            nc.scalar.activation(out=et[:], in_=ps[:], func=AF.Exp,
                                 bias=sbias[:])
            ets[jt] = et
            if pi > 1:
                emit_pv(pi - 2)
            # outproj fillers first (2 per step), then the steady global
            # P'-production pace of ~1.5 units per step
            for _ in range(2):
                u = next(fu, None)
                if u is not None:
                    u()
            drain_gq(2)
        emit_pv(NJT - 2)
        emit_pv(NJT - 1)
        for u in fu:
            u()
        # numerators -> ah (f32), denominators -> dram (f16)
        def finish():
            dt = dent.tile([1, 2, 512], F16, name="den", tag="den")
            for h in range(2):
                hs = slice(h * 64, h * 64 + 64)
                nc.vector.tensor_copy(out=_r(ah[hs, qs]),
                                      in_=_r(pot[h][0:64, :]))
                nc.vector.tensor_copy(out=dt[:, h, :], in_=pot[h][64:65, :])
            nc.gpsimd.dma_start(out=dens2[qc:qc + 1, :, :], in_=dt[:])
        return finish

    # ---- prologue: x DMAs || q proj || P'(0) || k/v proj ----
    # P'(0) needs only qT[:, 0:512] (q-pair 0), so its serial chain through
    # the pp slot starts immediately; proj units keep the PE busy between
    # the P' casts.
    p0 = p_units_for(0, act_casts=True)  # 3 ci + 1 write per qt -> 16
    p0i = iter(p0)
    q_pair_unit(0)
    q_pair_unit(1)
    next(p0i)()                     # qt0 ci0
    k_pair_unit(0)
    next(p0i)(); next(p0i)()        # qt0 ci1, ci2
    k_pair_unit(1)
    next(p0i)(); next(p0i)()        # qt0 write, qt1 ci0
    v_group_unit(0)
    next(p0i)(); next(p0i)()        # qt1 ci1, ci2
    v_group_unit(1)
    next(p0i)(); next(p0i)()        # qt1 write, qt2 ci0
    v_group_unit(2)
    next(p0i)(); next(p0i)()        # qt2 ci1, ci2
    v_group_unit(3)
    for u in p0i:                   # qt2 write, qt3 all
        u()
    skt = emit_skew_read(0)

    # ---- main loop over q-chunks in order 0, 3, 1, 2 ----
    # Each chunk's skew DMA is produced during an earlier iteration.  All
    # P'-production units live in ONE global queue drained at a steady,
    # DVE-sustainable pace (~1.5 units/step) across iterations, so the DVE
    # cast chain never bunches up in a single iteration.  outproj units for
    # the previous chunk are prepended (PE+ACT only, no DVE).
    QSEQ = [0, 3, 1, 2]
    skew_done = {}
    global_q = []
    for i, nqc in enumerate(QSEQ[1:]):
        global_q += p_units_for(nqc)
        global_q.append(lambda nqc=nqc: skew_done.update(
            {nqc: emit_skew_read(nqc)}))
    gqi = [0]

    def drain_gq(n):
        while n > 0 and gqi[0] < len(global_q):
            global_q[gqi[0]]()
            gqi[0] += 1
            n -= 1

    finish_prev = None
    for i, qc in enumerate(QSEQ):
        pqc = QSEQ[i - 1] if i > 0 else None
        # the next chunk's skew must be issued before its attn begins;
        # normally the pace suffices, this is a safety net
        if qc not in skew_done and i > 0:
            while qc not in skew_done and gqi[0] < len(global_q):
                drain_gq(1)
        o_fills = ([finish_prev] if finish_prev else []) + o_units_for(pqc)
        finish_prev = emit_attn(qc, skt if i == 0 else skew_done[qc],
                                o_fills, drain_gq)

    # tail: numerator copies + outproj for the last q-chunk
    finish_prev()
    for u in o_units_for(QSEQ[-1]):
        u()
    ctx.close()


_NC_CACHE = [None]


def _get_nc():
    if _NC_CACHE[0] is None:
        _NC_CACHE[0] = build_kernel()
    return _NC_CACHE[0]


def make_in_maps(x, Wq, Wkv, Wo, bo, rel_emb):
    bf16 = ml_dtypes.bfloat16
    xT = [np.ascontiguousarray(x[b].T).astype(bf16) for b in range(2)]
    cols = np.arange(W)
    idx = np.clip(1535 - cols, 0, 1024)
    relT = np.empty((128, W), np.float32)
    relT[0:64] = rel_emb[idx].T
    relT[64:128] = relT[0:64]
    relT = relT.astype(bf16)                       # reversed rel table
    relbc = np.empty((128, 256), np.float32)
    relbc[0:64, 0:128] = rel_emb[0][:, None]       # clamp-low value
    relbc[0:64, 128:256] = rel_emb[1024][:, None]  # clamp-high value
    relbc[64:128] = relbc[0:64]
    relbc = relbc.astype(bf16)
    ident = np.eye(128, dtype=np.float16)
    in_maps = []
    for c in range(8):
        b, hp = c // 4, c % 4
        cs = slice(hp * 128, hp * 128 + 128)
        wqkv = np.empty((4, 128, 3, 128), np.float32)
        for ci in range(4):
            rows = slice(ci * 128, ci * 128 + 128)
            wqkv[ci, :, 0, :] = Wq[rows, cs] / 8.0
            wqkv[ci, :, 1, :] = Wkv[rows, :512][:, cs]
            wqkv[ci, :, 2, :] = Wkv[rows, 512:][:, cs]
        in_maps.append({
            "xT": xT[b],
            "wqkv": wqkv.astype(bf16),
            "wo2": np.ascontiguousarray(Wo[cs, :]).astype(np.float32),
            "relT": relT,
            "relbc": relbc,
            "ident": ident,
        })
    return in_maps


def run(x, Wq, Wkv, Wo, bo, rel_emb, trace=False, trace_cores=None):
    nc = _get_nc()
    in_maps = make_in_maps(x, Wq, Wkv, Wo, bo, rel_emb)
    res = run_bass_kernel_spmd(nc, in_maps, core_ids=list(range(8)),
                               trace=trace, trace_cores=trace_cores)
    out = np.zeros((2, N, D), np.float32)
    for c in range(8):
        b = c // 4
        num = np.asarray(res.results[c]["outh"], np.float32)   # [2, N, D]
        den = np.asarray(res.results[c]["dens2"], np.float32)  # [4, 2, 512]
        for h in range(2):
            out[b] += num[h] / den[:, h, :].reshape(N)[:, None]
    out += np.asarray(bo, np.float32)[None, None, :]
    return out, res


def kernel(x, Wq, Wkv, Wo, bo, rel_emb):
    out, _ = run(np.asarray(x), np.asarray(Wq), np.asarray(Wkv),
                 np.asarray(Wo), np.asarray(bo), np.asarray(rel_emb))
    return out


# revision 20
# speedup vs baseline: 1.1521x; 1.1521x over previous
"""Trainium2 Bass kernel for nn_CausalAttention_76304388981436.

Full-input contract: kernel(**inputs) -> [2, 2048, 512] f32.

Sharding (8 cores, single SPMD program): core c = (batch b=c//4, head-pair
hp=c%4).  Each core computes attention for its 2 heads over the full 2048
sequence of its batch, producing per-head UNNORMALIZED projected numerators
outh[h] = (sum_j exp(S-11) vT)^T @ Wo[h-rows]  [2048, 512] f16 plus the
softmax denominators dens2 [4, 2, 512] f16; the host divides per head, sums
the 4 head-pair partials per batch and adds bo.

Device-side math per core (transposed-attention layout, heads packed at
partitions 0-63 / 64-127 of the PE array):
  qT[128i, 2048n] = wq2^T x^T   (wq2 pre-scaled by 1/8 on host; all inputs
                                 bf16 on host to halve DMA + run PE 1cyc/row)
  kT[128i, 2048n] = wk2^T x^T
  vta[128n, 16t, 2h, 65] = x wv2 (+ ones col 64 -> PV also produces denom)
  P' tiles [128q, 512s] = qT_h^T rel_embT_rev  (rel table pre-reversed on
      host; only the live s-chunks per q-chunk are computed/stored)
  PR dram per qc [2h, 512q, 2048s] fp16: bias[j,q] = PR[h, q, j-q+1023]
  skew read: ONE transposing DMA per (h, qc) with source AP
      [[2047, 512], [1, 128*njt]] (contiguous 2-3KB runs) -> SBUF
      [128j, njt, 512q]; h0 on the SP HWDGE queue, h1 on the ACT HWDGE
      queue so the two transfers overlap.
  S^T tile [128j, 1024(h0q|h1q)] = kT_h^T qT_h (both heads tile_position-
      paired); bias added via accumulating ident_f16 matmul (in-band) or
      clamp-value matmul (out-of-band) into the same PSUM group.
  expS = exp(S^T - 11)  one ACT op per [128, 1024] jt tile
  outT_h [65, 512q] += vta_h^T expS   (row 64 = denominator)
  outh[h] [2048q, 512] = numT_h^T wo2_h  (unnormalized, fp16 out)

Scheduling: per q-chunk the OUT-OF-BAND j-tiles run first so the skew DMA
(issued mid-previous-iteration right after the PR writes) overlaps compute.
P'(qc+1) units are front-loaded as PE filler inside attn(qc); outproj(qc-1)
trails at the iteration end.  Prologue pipelines x-chunk DMAs with q-proj,
P'(0), k/v-proj so the PE never sits idle waiting for input DMAs.
"""
import numpy as np
import ml_dtypes

import concourse.bass as bass
import concourse.mybir as mybir
import concourse.tile as tile
from concourse.bass_utils import run_bass_kernel_spmd

F32 = mybir.dt.float32
F32R = mybir.dt.float32r
F16 = mybir.dt.float16
BF16 = mybir.dt.bfloat16
AF = mybir.ActivationFunctionType

N = 2048          # sequence length
D = 512           # model dim
HD = 64           # head dim
NQC = 4           # q-chunks of 512
NJT = 16          # j-tiles of 128
W = 2048          # PR row width
SHIFT = -11.0     # exp(logit + SHIFT): keeps num/den in fp16 range

# per q-chunk: (jt_min, njt) of in-band j-tiles (-512 < A < 1152,
# A = 512*(qc+1) - 128*jt)
IN_BAND = {0: (0, 8), 1: (0, 12), 2: (4, 12), 3: (8, 8)}
# per q-chunk: live 512-wide s-chunks of PR (others never read)
LIVE_CI = {0: (1, 2, 3), 1: (0, 1, 2, 3), 2: (0, 1, 2, 3), 3: (0, 1, 2)}


def _split_multiwaits(nc):
    """This walrus build rejects >1 sync wait per instruction; split extra
    waits onto single-wait NoOps on the same engine just before it."""
    for func in nc.m.functions:
        for block in func.blocks:
            new_instrs = []
            for inst in block.instructions:
                si = inst.sync_info
                if si is not None and si.on_wait and len(si.on_wait) > 1:
                    waits = list(si.on_wait)
                    for w in waits[:-1]:
                        new_instrs.append(mybir.InstNoOp(
                            name=nc.get_next_instruction_name(),
                            engine=inst.engine,
                            bass_nofuse=True,
                            sync_info=mybir.SyncInfo(on_wait=[w], on_update=[]),
                        ))
                    si.on_wait = waits[-1:]
                new_instrs.append(inst)
            block.instructions = new_instrs


def _r(ap):
    return ap.bitcast(F32R)


def build_kernel():
    nc = bass.Bass("TRN2")
    xT = nc.dram_tensor("xT", [D, N], BF16, kind="ExternalInput")
    wqkv = nc.dram_tensor("wqkv", [4, 128, 3, 128], BF16, kind="ExternalInput")
    wo2 = nc.dram_tensor("wo2", [128, D], F32, kind="ExternalInput")
    relT = nc.dram_tensor("relT", [128, W], BF16, kind="ExternalInput")
    relbc = nc.dram_tensor("relbc", [128, 256], BF16, kind="ExternalInput")
    ident = nc.dram_tensor("ident", [128, 128], F16, kind="ExternalInput")
    outh = nc.dram_tensor("outh", [2, N, D], F16, kind="ExternalOutput")
    dens2 = nc.dram_tensor("dens2", [4, 2, 512], F16, kind="ExternalOutput")

    with tile.TileContext(nc) as tc:
        _build_body(nc, tc, xT, wqkv, wo2, relT, relbc, ident, outh, dens2)
    _split_multiwaits(nc)
    return nc


def _build_body(nc, tc, xT, wqkv, wo2, relT, relbc, ident, outh, dens2):
    from contextlib import ExitStack
    ctx = ExitStack()
    consts = ctx.enter_context(tc.tile_pool(name="consts", bufs=1))
    qkv = ctx.enter_context(tc.tile_pool(name="qkv", bufs=1))
    pc = ctx.enter_context(tc.tile_pool(name="pc", bufs=2))
    skew = ctx.enter_context(tc.tile_pool(name="skew", bufs=2))
    exps = ctx.enter_context(tc.tile_pool(name="exps", bufs=6))
    outc = ctx.enter_context(tc.tile_pool(name="outc", bufs=4))
    dent = ctx.enter_context(tc.tile_pool(name="dent", bufs=2))
    ring = ctx.enter_context(tc.tile_pool(name="ring", bufs=3, space="PSUM"))
    psum = ctx.enter_context(tc.tile_pool(name="psum", bufs=1, space="PSUM"))
    pdram = ctx.enter_context(tc.tile_pool(name="pdram", bufs=1, space="DRAM"))

    # ---- input DMAs (constants on the Pool SWDGE queue, x on SP/ACT) ----
    # order by first use: wqkv (q proj) -> relT (P'0) -> relbc/ident (attn0)
    # -> wo (outproj, attn1)
    swqkv = consts.tile([128, 4, 3, 128], BF16, name="wqkv", tag="wqkv")
    nc.gpsimd.dma_start(
        out=swqkv[:],
        in_=wqkv.rearrange("c p k m -> p c k m"))
    srelT = consts.tile([128, W], BF16, name="relT", tag="relT")
    nc.gpsimd.dma_start(out=srelT[:], in_=relT[:, :])
    srelbc = consts.tile([128, 256], BF16, name="relbc", tag="relbc")
    nc.gpsimd.dma_start(out=srelbc[:], in_=relbc[:, :])
    sident = consts.tile([128, 128], F16, name="ident", tag="ident")
    nc.gpsimd.dma_start(out=sident[:], in_=ident[:, :])
    swo = consts.tile([128, D], F32, name="wo", tag="wo")
    nc.gpsimd.dma_start(out=_r(swo[:]), in_=_r(wo2[:, :]))
    sbias = consts.tile([128, 1], F32, name="sbias", tag="sbias")
    nc.gpsimd.memset(sbias[:], SHIFT)

    sxT = [consts.tile([128, N], BF16, name=f"xT{i}", tag=f"xT{i}")
           for i in range(4)]
    for half in range(2):
        ns = slice(half * 1024, half * 1024 + 1024)
        for i in range(4):
            eng = nc.sync if i % 2 == 0 else nc.scalar
            eng.dma_start(out=sxT[i][:, ns], in_=xT[i * 128:(i + 1) * 128, ns])

    # ---- SBUF working tensors ----
    qT = qkv.tile([128, N], BF16, name="qT", tag="qT")
    kT = qkv.tile([128, N], BF16, name="kT", tag="kT")
    # v with ones column per (t, head): [n-in-tile, t, h, d|1]
    vta = qkv.tile([128, NJT, 2, 65], BF16, name="vta", tag="vta")
    nc.gpsimd.memset(vta[:, :, :, 64:65], 1.0)
    # ah holds both heads' unnormalized numerators: rows 0-63 h0, 64-127 h1
    ah = qkv.tile([128, N], F32, name="ah", tag="ah")

    # PSUM: one FIFO ring of 3 x [128,1024] (6 banks) shared by ALL
    # paired-matmul outputs (S tiles, P' units, proj, outproj) + 2 banks
    # for the PV accumulators = 8 banks.  The ring decouples each unit's
    # matmuls from its own consumer (exp/cast) by ~2 units of pipeline
    # distance and keeps h0/h1 tile_position pairing intact.
    def mm_tile():
        return ring.tile([128, 1024], F32, name="mm", tag="mm")

    # ---- per-qc PR dram tensors [2h, 512q, W] ----
    prd = {qc: pdram.tile([2, 512, W], F16, name=f"pr{qc}", tag=f"pr{qc}")
           for qc in range(NQC)}

    # ---- projection units ----
    def q_pair_unit(pair):
        ps = mm_tile()
        for nl in range(2):
            nch = 2 * pair + nl
            ns = slice(nch * 512, nch * 512 + 512)
            for c in range(4):
                nc.tensor.matmul(ps[:, nl * 512:nl * 512 + 512],
                                 swqkv[:, c, 0, :], sxT[c][:, ns],
                                 start=(c == 0), stop=(c == 3))
        nc.vector.tensor_copy(out=qT[:, pair * 1024:pair * 1024 + 1024],
                              in_=ps[:])

    def k_pair_unit(pair):
        ps = mm_tile()
        for nl in range(2):
            nch = 2 * pair + nl
            ns = slice(nch * 512, nch * 512 + 512)
            for c in range(4):
                nc.tensor.matmul(ps[:, nl * 512:nl * 512 + 512],
                                 swqkv[:, c, 1, :], sxT[c][:, ns],
                                 start=(c == 0), stop=(c == 3))
        nc.vector.tensor_copy(out=kT[:, pair * 1024:pair * 1024 + 1024],
                              in_=ps[:])

    def v_group_unit(g):
        # tiles t = 4g .. 4g+3 -> one [128, 512] half of a ring tile
        ps = mm_tile()
        half = ps[:, 0:512]
        for tl in range(4):
            t = 4 * g + tl
            nst = slice(t * 128, t * 128 + 128)
            for c in range(4):
                nc.tensor.matmul(half[:, tl * 128:tl * 128 + 128],
                                 sxT[c][:, nst], swqkv[:, c, 2, :],
                                 start=(c == 0), stop=(c == 3))
        nc.vector.tensor_copy(
            out=vta[:, 4 * g:4 * g + 4, :, 0:64],
            in_=half.rearrange("p (t h d) -> p t h d", t=4, h=2))

    # ---- P' production units for q-chunk qc ----
    def p_units_for(qc, act_casts=False):
        """Fine-grained closures producing P'(qc): one per (qt, ci) matmul+
        copy step, plus one per qt for the PR write DMA.  act_casts=True
        alternates the PSUM->SBUF casts between DVE and ACT — only safe in
        the prologue where the ACT engine has no exp stream to block."""
        if qc is None or qc >= NQC:
            return []
        units = []
        jt_min, njt = IN_BAND[qc]
        off = 128 * jt_min - 512 * qc + 1023
        state = {}
        nu = 0
        for qt_local in range(4):
            qt = 4 * qc + qt_local
            # exact band window for this qt's rows, 256-aligned: the skew
            # parallelogram reads cols [off-a, off-a+128*njt) for each row a
            amin, amax = 128 * qt_local, 128 * qt_local + 127
            lo = max(0, (off - amax) // 256 * 256)
            hi = min(W, -((off - amin + 128 * njt) // -256) * 256)

            w0s = list(range(lo, hi, 512))
            for k, w0 in enumerate(w0s):
                cw = min(512, hi - w0)
                nu += 1
                def do_ci(qt=qt, w0=w0, cw=cw, first=(k == 0),
                          on_act=(act_casts and nu % 2 == 0)):
                    if first:
                        state[qt] = pc.tile([128, 2, W], F16, name="pct",
                                            tag="pct")
                    qs = slice(qt * 128, qt * 128 + 128)
                    ps = mm_tile()
                    for h in range(2):
                        hs = slice(h * 64, h * 64 + 64)
                        nc.tensor.matmul(ps[:, h * 512:h * 512 + cw],
                                         qT[hs, qs], srelT[hs, w0:w0 + cw],
                                         start=True, stop=True,
                                         tile_position=(h * 64, 0))
                    hv = ps[:].rearrange("p (h s) -> p h s", h=2)[:, :, 0:cw]
                    if on_act:
                        nc.scalar.activation(out=state[qt][:, :, w0:w0 + cw],
                                             in_=hv, func=AF.Copy)
                    else:
                        nc.vector.tensor_copy(out=state[qt][:, :, w0:w0 + cw],
                                              in_=hv)

                units.append(do_ci)

            def write_qt(qt=qt, qt_local=qt_local, lo=lo, hi=hi):
                rows = slice(qt_local * 128, qt_local * 128 + 128)
                pct = state[qt]
                nc.gpsimd.dma_start(
                    out=prd[qc][:, rows, lo:hi].rearrange("h r w -> r h w"),
                    in_=pct[:, :, lo:hi])

            units.append(write_qt)
        return units

    def emit_skew_read(qc):
        """Transposing DMAs covering all in-band j-tiles.  All on the SP
        queue: concurrent XBAR transposes on different queues corrupt each
        other (verified on HW).  Each head is split in two halves ordered
        (h0a, h1a, h0b, h1b) so the first in-band tiles of both heads are
        available after ~half the total transfer time."""
        jt_min, njt = IN_BAND[qc]
        t = prd[qc]
        tiles = {}
        for h in range(2):
            tiles[h] = skew.tile([128, 12, 512], F16, name="skt",
                                 tag=f"skt{h}")
        for t0 in range(0, njt, 4):
            tn = min(4, njt - t0)
            for h in range(2):
                src = bass.AP(tensor=t.tensor,
                              offset=t.offset + h * 512 * W + 128 * t0
                              + 128 * jt_min - 512 * qc + 1023,
                              ap=[[2047, 512], [1, 128 * tn]])
                nc.sync.dma_start(out=tiles[h][:, t0:t0 + tn, :], in_=src,
                                  transpose=True)
        return tiles

    # ---- output projection units for q-chunk qc (unnormalized) ----
    def o_units_for(qc):
        if qc is None or qc < 0:
            return []
        units = []
        for qt_local in range(4):
            qt = 4 * qc + qt_local

            def do_o(qt=qt):
                qs = slice(qt * 128, qt * 128 + 128)
                ps = mm_tile()
                for h in range(2):
                    hs = slice(h * 64, h * 64 + 64)
                    nc.tensor.matmul(ps[:, h * 512:h * 512 + 512],
                                     _r(ah[hs, qs]), _r(swo[hs, :]),
                                     start=True, stop=True,
                                     tile_position=(h * 64, 0))
                ot = outc.tile([128, 1024], F16, name="oc", tag="oc")
                nc.scalar.activation(out=ot[:], in_=ps[:], func=AF.Copy)
                nc.gpsimd.dma_start(
                    out=outh[:, qs, :].rearrange("h q d -> q h d"),
                    in_=ot[:].rearrange("p (h d) -> p h d", h=2))

            units.append(do_o)
        return units

    # ---- attention for one q-chunk ----
    def emit_attn(qc, skt, o_fills, drain_gq):
        jt_min, njt = IN_BAND[qc]
        in_band = lambda jt: jt_min <= jt < jt_min + njt
        jts = [jt for jt in range(NJT) if not in_band(jt)] + \
              [jt for jt in range(NJT) if in_band(jt)]
        qs = slice(qc * 512, qc * 512 + 512)
        pot = [psum.tile([65, 512], F32, name="po", tag=f"po{h}")
               for h in range(2)]
        ets = {}
        fu = iter(o_fills)

        def emit_pv(pi):
            jt = jts[pi]
            for h in range(2):
                nc.tensor.matmul(pot[h][:], vta[:, jt, h, :],
                                 ets[jt][:, h * 512:h * 512 + 512],
                                 start=(pi == 0), stop=(pi == NJT - 1))

        for pi, jt in enumerate(jts):
            js = slice(jt * 128, jt * 128 + 128)
            ps = mm_tile()
            for h in range(2):
                hs = slice(h * 64, h * 64 + 64)
                nc.tensor.matmul(ps[:, h * 512:h * 512 + 512],
                                 kT[hs, js], qT[hs, qs],
                                 start=True, stop=False,
                                 tile_position=(h * 64, 0))
            if in_band(jt):
                for h in range(2):
                    nc.tensor.matmul(ps[:, h * 512:h * 512 + 512],
                                     sident[:], skt[h][:, jt - jt_min, :],
                                     start=False, stop=True)
            else:
                A = qc * 512 + 512 - 128 * jt
                bc = 0 if A <= -512 else 128
                for h in range(2):
                    hs = slice(h * 64, h * 64 + 64)
                    nc.tensor.matmul(ps[:, h * 512:h * 512 + 512],
                                     srelbc[hs, bc:bc + 128], qT[hs, qs],
                                     start=False, stop=True,
                                     tile_position=(h * 64, 0))
            et = exps.tile([128, 1024], BF16, name="expS", tag="expS")
            nc.scalar.activation(out=et[:], in_=ps[:], func=AF.Exp,
                                 bias=sbias[:])
            ets[jt] = et
            if pi > 1:
                emit_pv(pi - 2)
            # outproj fillers first (2 per step), then the steady global
            # P'-production pace of ~1.5 units per step
            for _ in range(2):
                u = next(fu, None)
                if u is not None:
                    u()
            drain_gq(2)
        emit_pv(NJT - 2)
        emit_pv(NJT - 1)
        for u in fu:
            u()
        # numerators -> ah (f32), denominators -> dram (f16)
        def finish():
            dt = dent.tile([1, 2, 512], F16, name="den", tag="den")
            for h in range(2):
                hs = slice(h * 64, h * 64 + 64)
                nc.vector.tensor_copy(out=_r(ah[hs, qs]),
                                      in_=_r(pot[h][0:64, :]))
                nc.vector.tensor_copy(out=dt[:, h, :], in_=pot[h][64:65, :])
            nc.gpsimd.dma_start(out=dens2[qc:qc + 1, :, :], in_=dt[:])
        return finish

    # ---- prologue: x DMAs || q proj || P'(0) || k/v proj ----
    # P'(0) needs only qT[:, 0:512] (q-pair 0), so its serial chain through
    # the pp slot starts immediately; proj units keep the PE busy between
    # the P' casts.
    p0 = p_units_for(0, act_casts=True)  # 3 ci + 1 write per qt -> 16
    p0i = iter(p0)
    q_pair_unit(0)
    q_pair_unit(1)
    next(p0i)()                     # qt0 ci0
    k_pair_unit(0)
    next(p0i)(); next(p0i)()        # qt0 ci1, ci2
    k_pair_unit(1)
    next(p0i)(); next(p0i)()        # qt0 write, qt1 ci0
    v_group_unit(0)
    next(p0i)(); next(p0i)()        # qt1 ci1, ci2
    v_group_unit(1)
    next(p0i)(); next(p0i)()        # qt1 write, qt2 ci0
    v_group_unit(2)
    next(p0i)(); next(p0i)()        # qt2 ci1, ci2
    v_group_unit(3)
    for u in p0i:                   # qt2 write, qt3 all
        u()
    skt = emit_skew_read(0)

    # ---- main loop over q-chunks in order 0, 3, 1, 2 ----
    # Each chunk's skew DMA is produced during an earlier iteration.  All
    # P'-production units live in ONE global queue drained at a steady,
    # DVE-sustainable pace (~1.5 units/step) across iterations, so the DVE
    # cast chain never bunches up in a single iteration.  outproj units for
    # the previous chunk are prepended (PE+ACT only, no DVE).
    QSEQ = [0, 3, 1, 2]
    skew_done = {}
    global_q = []
    for i, nqc in enumerate(QSEQ[1:]):
        global_q += p_units_for(nqc)
        global_q.append(lambda nqc=nqc: skew_done.update(
            {nqc: emit_skew_read(nqc)}))
    gqi = [0]

    def drain_gq(n):
        while n > 0 and gqi[0] < len(global_q):
            global_q[gqi[0]]()
            gqi[0] += 1
            n -= 1

    finish_prev = None
    for i, qc in enumerate(QSEQ):
        pqc = QSEQ[i - 1] if i > 0 else None
        # the next chunk's skew must be issued before its attn begins;
        # normally the pace suffices, this is a safety net
        if qc not in skew_done and i > 0:
            while qc not in skew_done and gqi[0] < len(global_q):
                drain_gq(1)
        o_fills = ([finish_prev] if finish_prev else []) + o_units_for(pqc)
        finish_prev = emit_attn(qc, skt if i == 0 else skew_done[qc],
                                o_fills, drain_gq)

    # tail: numerator copies + outproj for the last q-chunk
    finish_prev()
    for u in o_units_for(QSEQ[-1]):
        u()
    ctx.close()


_NC_CACHE = [None]


def _get_nc():
    if _NC_CACHE[0] is None:
        _NC_CACHE[0] = build_kernel()
    return _NC_CACHE[0]


def make_in_maps(x, Wq, Wkv, Wo, bo, rel_emb):
    bf16 = ml_dtypes.bfloat16
    xT = [np.ascontiguousarray(x[b].T).astype(bf16) for b in range(2)]
    cols = np.arange(W)
    idx = np.clip(1535 - cols, 0, 1024)
    relT = np.empty((128, W), np.float32)
    relT[0:64] = rel_emb[idx].T
    relT[64:128] = relT[0:64]
    relT = relT.astype(bf16)                       # reversed rel table
    relbc = np.empty((128, 256), np.float32)
    relbc[0:64, 0:128] = rel_emb[0][:, None]       # clamp-low value
    relbc[0:64, 128:256] = rel_emb[1024][:, None]  # clamp-high value
    relbc[64:128] = relbc[0:64]
    relbc = relbc.astype(bf16)
    ident = np.eye(128, dtype=np.float16)
    in_maps = []
    for c in range(8):
        b, hp = c // 4, c % 4
        cs = slice(hp * 128, hp * 128 + 128)
        wqkv = np.empty((4, 128, 3, 128), np.float32)
        for ci in range(4):
            rows = slice(ci * 128, ci * 128 + 128)
            wqkv[ci, :, 0, :] = Wq[rows, cs] / 8.0
            wqkv[ci, :, 1, :] = Wkv[rows, :512][:, cs]
            wqkv[ci, :, 2, :] = Wkv[rows, 512:][:, cs]
        in_maps.append({
            "xT": xT[b],
            "wqkv": wqkv.astype(bf16),
            "wo2": np.ascontiguousarray(Wo[cs, :]).astype(np.float32),
            "relT": relT,
            "relbc": relbc,
            "ident": ident,
        })
    return in_maps


def run(x, Wq, Wkv, Wo, bo, rel_emb, trace=False, trace_cores=None):
    nc = _get_nc()
    in_maps = make_in_maps(x, Wq, Wkv, Wo, bo, rel_emb)
    res = run_bass_kernel_spmd(nc, in_maps, core_ids=list(range(8)),
                               trace=trace, trace_cores=trace_cores)
    out = np.zeros((2, N, D), np.float32)
    for c in range(8):
        b = c // 4
        num = np.asarray(res.results[c]["outh"], np.float32)   # [2, N, D]
        den = np.asarray(res.results[c]["dens2"], np.float32)  # [4, 2, 512]
        for h in range(2):
            out[b] += num[h] / den[:, h, :].reshape(N)[:, None]
    out += np.asarray(bo, np.float32)[None, None, :]
    return out, res


def kernel(x, Wq, Wkv, Wo, bo, rel_emb):
    out, _ = run(np.asarray(x), np.asarray(Wq), np.asarray(Wkv),
                 np.asarray(Wo), np.asarray(bo), np.asarray(rel_emb))
    return out


# revision 21
# speedup vs baseline: 1.2760x; 1.1076x over previous
"""Trainium2 Bass kernel for nn_CausalAttention_76304388981436.

Full-input contract: kernel(**inputs) -> [2, 2048, 512] f32.

Sharding (8 cores, single SPMD program): core c = (batch b=c//4, head-pair
hp=c%4).  Each core computes attention for its 2 heads over the full 2048
sequence of its batch, producing per-head UNNORMALIZED projected numerators
outh[h] = (sum_j exp(S-11) vT)^T @ Wo[h-rows]  [2048, 512] f16 plus the
softmax denominators dens2 [4, 2, 512] f16; the host divides per head, sums
the 4 head-pair partials per batch and adds bo.

Device-side math per core (transposed-attention layout, heads packed at
partitions 0-63 / 64-127 of the PE array):
  qT[128i, 2048n] = wq2^T x^T   (wq2 pre-scaled by 1/8 on host; all inputs
                                 bf16 on host to halve DMA + run PE 1cyc/row)
  kT[128i, 2048n] = wk2^T x^T
  vta[128n, 16t, 2h, 65] = x wv2 (+ ones col 64 -> PV also produces denom)
  P' tiles [128q, 512s] = qT_h^T rel_embT_rev  (rel table pre-reversed on
      host; only the live s-chunks per q-chunk are computed/stored)
  PR dram per qc [2h, 512q, 2048s] fp16: bias[j,q] = PR[h, q, j-q+1023]
  skew read: ONE transposing DMA per (h, qc) with source AP
      [[2047, 512], [1, 128*njt]] (contiguous 2-3KB runs) -> SBUF
      [128j, njt, 512q]; h0 on the SP HWDGE queue, h1 on the ACT HWDGE
      queue so the two transfers overlap.
  S^T tile [128j, 1024(h0q|h1q)] = kT_h^T qT_h (both heads tile_position-
      paired); bias added via accumulating ident_f16 matmul (in-band) or
      clamp-value matmul (out-of-band) into the same PSUM group.
  expS = exp(S^T - 11)  one ACT op per [128, 1024] jt tile
  outT_h [65, 512q] += vta_h^T expS   (row 64 = denominator)
  outh[h] [2048q, 512] = numT_h^T wo2_h  (unnormalized, fp16 out)

Scheduling: per q-chunk the OUT-OF-BAND j-tiles run first so the skew DMA
(issued mid-previous-iteration right after the PR writes) overlaps compute.
P'(qc+1) units are front-loaded as PE filler inside attn(qc); outproj(qc-1)
trails at the iteration end.  Prologue pipelines x-chunk DMAs with q-proj,
P'(0), k/v-proj so the PE never sits idle waiting for input DMAs.
"""
import numpy as np
import ml_dtypes

import concourse.bass as bass
import concourse.mybir as mybir
import concourse.tile as tile
from concourse.bass_utils import run_bass_kernel_spmd

F32 = mybir.dt.float32
F32R = mybir.dt.float32r
F16 = mybir.dt.float16
BF16 = mybir.dt.bfloat16
AF = mybir.ActivationFunctionType

N = 2048          # sequence length
D = 512           # model dim
HD = 64           # head dim
NQC = 4           # q-chunks of 512
NJT = 16          # j-tiles of 128
W = 2048          # PR row width
SHIFT = -11.0     # exp(logit + SHIFT): keeps num/den in fp16 range

# per q-chunk: (jt_min, njt) of in-band j-tiles (-512 < A < 1152,
# A = 512*(qc+1) - 128*jt)
IN_BAND = {0: (0, 8), 1: (0, 12), 2: (4, 12), 3: (8, 8)}
# per q-chunk: live 512-wide s-chunks of PR (others never read)
LIVE_CI = {0: (1, 2, 3), 1: (0, 1, 2, 3), 2: (0, 1, 2, 3), 3: (0, 1, 2)}


def _split_multiwaits(nc):
    """This walrus build rejects >1 sync wait per instruction; split extra
    waits onto single-wait NoOps on the same engine just before it."""
    for func in nc.m.functions:
        for block in func.blocks:
            new_instrs = []
            for inst in block.instructions:
                si = inst.sync_info
                if si is not None and si.on_wait and len(si.on_wait) > 1:
                    waits = list(si.on_wait)
                    for w in waits[:-1]:
                        new_instrs.append(mybir.InstNoOp(
                            name=nc.get_next_instruction_name(),
                            engine=inst.engine,
                            bass_nofuse=True,
                            sync_info=mybir.SyncInfo(on_wait=[w], on_update=[]),
                        ))
                    si.on_wait = waits[-1:]
                new_instrs.append(inst)
            block.instructions = new_instrs


def _r(ap):
    return ap.bitcast(F32R)


def build_kernel():
    nc = bass.Bass("TRN2")
    xT = nc.dram_tensor("xT", [D, N], BF16, kind="ExternalInput")
    wqkv = nc.dram_tensor("wqkv", [4, 128, 3, 128], BF16, kind="ExternalInput")
    wo2 = nc.dram_tensor("wo2", [128, D], F32, kind="ExternalInput")
    relT = nc.dram_tensor("relT", [128, W], BF16, kind="ExternalInput")
    relbc = nc.dram_tensor("relbc", [128, 256], BF16, kind="ExternalInput")
    ident = nc.dram_tensor("ident", [128, 128], F16, kind="ExternalInput")
    outh = nc.dram_tensor("outh", [2, N, D], F16, kind="ExternalOutput")
    dens2 = nc.dram_tensor("dens2", [4, 2, 512], F16, kind="ExternalOutput")

    with tile.TileContext(nc) as tc:
        _build_body(nc, tc, xT, wqkv, wo2, relT, relbc, ident, outh, dens2)
    _split_multiwaits(nc)
    return nc


def _build_body(nc, tc, xT, wqkv, wo2, relT, relbc, ident, outh, dens2):
    from contextlib import ExitStack
    ctx = ExitStack()
    consts = ctx.enter_context(tc.tile_pool(name="consts", bufs=1))
    qkv = ctx.enter_context(tc.tile_pool(name="qkv", bufs=1))
    pc = ctx.enter_context(tc.tile_pool(name="pc", bufs=2))
    skew = ctx.enter_context(tc.tile_pool(name="skew", bufs=2))
    exps = ctx.enter_context(tc.tile_pool(name="exps", bufs=6))
    outc = ctx.enter_context(tc.tile_pool(name="outc", bufs=4))
    dent = ctx.enter_context(tc.tile_pool(name="dent", bufs=2))
    ring = ctx.enter_context(tc.tile_pool(name="ring", bufs=3, space="PSUM"))
    psum = ctx.enter_context(tc.tile_pool(name="psum", bufs=1, space="PSUM"))
    pdram = ctx.enter_context(tc.tile_pool(name="pdram", bufs=1, space="DRAM"))

    # ---- input DMAs (constants on the Pool SWDGE queue, x on SP/ACT) ----
    # order by first use: wqkv (q proj) -> relT (P'0) -> relbc/ident (attn0)
    # -> wo (outproj, attn1)
    swqkv = consts.tile([128, 4, 3, 128], BF16, name="wqkv", tag="wqkv")
    nc.gpsimd.dma_start(
        out=swqkv[:],
        in_=wqkv.rearrange("c p k m -> p c k m"))
    srelT = consts.tile([128, W], BF16, name="relT", tag="relT")
    nc.gpsimd.dma_start(out=srelT[:], in_=relT[:, :])
    srelbc = consts.tile([128, 256], BF16, name="relbc", tag="relbc")
    nc.gpsimd.dma_start(out=srelbc[:], in_=relbc[:, :])
    sident = consts.tile([128, 128], F16, name="ident", tag="ident")
    nc.gpsimd.dma_start(out=sident[:], in_=ident[:, :])
    swo = consts.tile([128, D], F32, name="wo", tag="wo")
    nc.gpsimd.dma_start(out=_r(swo[:]), in_=_r(wo2[:, :]))
    sbias = consts.tile([128, 1], F32, name="sbias", tag="sbias")
    nc.gpsimd.memset(sbias[:], SHIFT)

    sxT = [consts.tile([128, N], BF16, name=f"xT{i}", tag=f"xT{i}")
           for i in range(4)]
    for half in range(2):
        ns = slice(half * 1024, half * 1024 + 1024)
        for i in range(4):
            eng = nc.sync if i % 2 == 0 else nc.scalar
            eng.dma_start(out=sxT[i][:, ns], in_=xT[i * 128:(i + 1) * 128, ns])

    # ---- SBUF working tensors ----
    qT = qkv.tile([128, N], BF16, name="qT", tag="qT")
    kT = qkv.tile([128, N], BF16, name="kT", tag="kT")
    # v with ones column per (t, head): [n-in-tile, t, h, d|1]
    vta = qkv.tile([128, NJT, 2, 65], BF16, name="vta", tag="vta")
    nc.gpsimd.memset(vta[:, :, :, 64:65], 1.0)
    # ah holds both heads' unnormalized numerators: rows 0-63 h0, 64-127 h1
    ah = qkv.tile([128, N], F32, name="ah", tag="ah")

    # PSUM: one FIFO ring of 3 x [128,1024] (6 banks) shared by ALL
    # paired-matmul outputs (S tiles, P' units, proj, outproj) + 2 banks
    # for the PV accumulators = 8 banks.  The ring decouples each unit's
    # matmuls from its own consumer (exp/cast) by ~2 units of pipeline
    # distance and keeps h0/h1 tile_position pairing intact.
    def mm_tile():
        return ring.tile([128, 1024], F32, name="mm", tag="mm")

    # ---- per-qc PR dram tensors [2h, 512q, W] ----
    prd = {qc: pdram.tile([2, 512, W], F16, name=f"pr{qc}", tag=f"pr{qc}")
           for qc in range(NQC)}

    # ---- projection units ----
    def q_pair_unit(pair):
        ps = mm_tile()
        for nl in range(2):
            nch = 2 * pair + nl
            ns = slice(nch * 512, nch * 512 + 512)
            for c in range(4):
                nc.tensor.matmul(ps[:, nl * 512:nl * 512 + 512],
                                 swqkv[:, c, 0, :], sxT[c][:, ns],
                                 start=(c == 0), stop=(c == 3))
        nc.vector.tensor_copy(out=qT[:, pair * 1024:pair * 1024 + 1024],
                              in_=ps[:])

    def k_pair_unit(pair):
        ps = mm_tile()
        for nl in range(2):
            nch = 2 * pair + nl
            ns = slice(nch * 512, nch * 512 + 512)
            for c in range(4):
                nc.tensor.matmul(ps[:, nl * 512:nl * 512 + 512],
                                 swqkv[:, c, 1, :], sxT[c][:, ns],
                                 start=(c == 0), stop=(c == 3))
        nc.vector.tensor_copy(out=kT[:, pair * 1024:pair * 1024 + 1024],
                              in_=ps[:])

    def v_group_unit(g):
        # tiles t = 4g .. 4g+3 -> one [128, 512] half of a ring tile
        ps = mm_tile()
        half = ps[:, 0:512]
        for tl in range(4):
            t = 4 * g + tl
            nst = slice(t * 128, t * 128 + 128)
            for c in range(4):
                nc.tensor.matmul(half[:, tl * 128:tl * 128 + 128],
                                 sxT[c][:, nst], swqkv[:, c, 2, :],
                                 start=(c == 0), stop=(c == 3))
        nc.vector.tensor_copy(
            out=vta[:, 4 * g:4 * g + 4, :, 0:64],
            in_=half.rearrange("p (t h d) -> p t h d", t=4, h=2))

    # ---- P' production units for q-chunk qc ----
    def p_units_for(qc, act_casts=False):
        """Fine-grained closures producing P'(qc): one per (qt, ci) matmul+
        copy step, plus one per qt for the PR write DMA.  act_casts=True
        alternates the PSUM->SBUF casts between DVE and ACT — only safe in
        the prologue where the ACT engine has no exp stream to block."""
        if qc is None or qc >= NQC:
            return []
        units = []
        jt_min, njt = IN_BAND[qc]
        off = 128 * jt_min - 512 * qc + 1023
        state = {}
        nu = 0
        for qt_local in range(4):
            qt = 4 * qc + qt_local
            # exact band window for this qt's rows, 256-aligned: the skew
            # parallelogram reads cols [off-a, off-a+128*njt) for each row a
            amin, amax = 128 * qt_local, 128 * qt_local + 127
            lo = max(0, (off - amax) // 256 * 256)
            hi = min(W, -((off - amin + 128 * njt) // -256) * 256)

            w0s = list(range(lo, hi, 512))
            for k, w0 in enumerate(w0s):
                cw = min(512, hi - w0)
                nu += 1
                def do_ci(qt=qt, w0=w0, cw=cw, first=(k == 0),
                          on_act=(act_casts and nu % 2 == 0)):
                    if first:
                        state[qt] = pc.tile([128, 2, W], F16, name="pct",
                                            tag="pct")
                    qs = slice(qt * 128, qt * 128 + 128)
                    ps = mm_tile()
                    for h in range(2):
                        hs = slice(h * 64, h * 64 + 64)
                        nc.tensor.matmul(ps[:, h * 512:h * 512 + cw],
                                         qT[hs, qs], srelT[hs, w0:w0 + cw],
                                         start=True, stop=True,
                                         tile_position=(h * 64, 0))
                    hv = ps[:].rearrange("p (h s) -> p h s", h=2)[:, :, 0:cw]
                    if on_act:
                        nc.scalar.activation(out=state[qt][:, :, w0:w0 + cw],
                                             in_=hv, func=AF.Copy)
                    else:
                        nc.vector.tensor_copy(out=state[qt][:, :, w0:w0 + cw],
                                              in_=hv)

                units.append(do_ci)

            def write_qt(qt=qt, qt_local=qt_local, lo=lo, hi=hi):
                rows = slice(qt_local * 128, qt_local * 128 + 128)
                pct = state[qt]
                nc.gpsimd.dma_start(
                    out=prd[qc][:, rows, lo:hi].rearrange("h r w -> r h w"),
                    in_=pct[:, :, lo:hi])

            units.append(write_qt)
        return units

    def emit_skew_read(qc):
        """Transposing DMAs covering all in-band j-tiles.  All on the SP
        queue: concurrent XBAR transposes on different queues corrupt each
        other (verified on HW).  Each head is split in two halves ordered
        (h0a, h1a, h0b, h1b) so the first in-band tiles of both heads are
        available after ~half the total transfer time."""
        jt_min, njt = IN_BAND[qc]
        t = prd[qc]
        tiles = {}
        for h in range(2):
            tiles[h] = skew.tile([128, 12, 512], F16, name="skt",
                                 tag=f"skt{h}")
        na = njt // 2
        for t0 in (0, na):
            tn = na if t0 == 0 else njt - na
            for h in range(2):
                src = bass.AP(tensor=t.tensor,
                              offset=t.offset + h * 512 * W + 128 * t0
                              + 128 * jt_min - 512 * qc + 1023,
                              ap=[[2047, 512], [1, 128 * tn]])
                nc.sync.dma_start(out=tiles[h][:, t0:t0 + tn, :], in_=src,
                                  transpose=True)
        return tiles

    # ---- output projection units for q-chunk qc (unnormalized) ----
    def o_units_for(qc):
        if qc is None or qc < 0:
            return []
        units = []
        for qt_local in range(4):
            qt = 4 * qc + qt_local

            def do_o(qt=qt):
                qs = slice(qt * 128, qt * 128 + 128)
                ps = mm_tile()
                for h in range(2):
                    hs = slice(h * 64, h * 64 + 64)
                    nc.tensor.matmul(ps[:, h * 512:h * 512 + 512],
                                     _r(ah[hs, qs]), _r(swo[hs, :]),
                                     start=True, stop=True,
                                     tile_position=(h * 64, 0))
                ot = outc.tile([128, 1024], F16, name="oc", tag="oc")
                nc.scalar.activation(out=ot[:], in_=ps[:], func=AF.Copy)
                nc.sync.dma_start(
                    out=outh[:, qs, :].rearrange("h q d -> q h d"),
                    in_=ot[:].rearrange("p (h d) -> p h d", h=2))

            units.append(do_o)
        return units

    # ---- attention for one q-chunk ----
    def emit_attn(qc, skt, o_fills, drain_gq):
        jt_min, njt = IN_BAND[qc]
        in_band = lambda jt: jt_min <= jt < jt_min + njt
        jts = [jt for jt in range(NJT) if not in_band(jt)] + \
              [jt for jt in range(NJT) if in_band(jt)]
        qs = slice(qc * 512, qc * 512 + 512)
        pot = [psum.tile([65, 512], F32, name="po", tag=f"po{h}")
               for h in range(2)]
        ets = {}
        fu = iter(o_fills)

        def emit_pv(pi):
            jt = jts[pi]
            for h in range(2):
                nc.tensor.matmul(pot[h][:], vta[:, jt, h, :],
                                 ets[jt][:, h * 512:h * 512 + 512],
                                 start=(pi == 0), stop=(pi == NJT - 1))

        for pi, jt in enumerate(jts):
            js = slice(jt * 128, jt * 128 + 128)
            ps = mm_tile()
            for h in range(2):
                hs = slice(h * 64, h * 64 + 64)
                nc.tensor.matmul(ps[:, h * 512:h * 512 + 512],
                                 kT[hs, js], qT[hs, qs],
                                 start=True, stop=False,
                                 tile_position=(h * 64, 0))
            if in_band(jt):
                for h in range(2):
                    nc.tensor.matmul(ps[:, h * 512:h * 512 + 512],
                                     sident[:], skt[h][:, jt - jt_min, :],
                                     start=False, stop=True)
            else:
                A = qc * 512 + 512 - 128 * jt
                bc = 0 if A <= -512 else 128
                for h in range(2):
                    hs = slice(h * 64, h * 64 + 64)
                    nc.tensor.matmul(ps[:, h * 512:h * 512 + 512],
                                     srelbc[hs, bc:bc + 128], qT[hs, qs],
                                     start=False, stop=True,
                                     tile_position=(h * 64, 0))
            et = exps.tile([128, 1024], BF16, name="expS", tag="expS")
            nc.scalar.activation(out=et[:], in_=ps[:], func=AF.Exp,
                                 bias=sbias[:])
            ets[jt] = et
            if pi > 1:
                emit_pv(pi - 2)
            # outproj fillers first (2 per step), then the steady global
            # P'-production pace of ~1.5 units per step
            for _ in range(2):
                u = next(fu, None)
                if u is not None:
                    u()
            drain_gq(2 if pi % 2 == 0 else 1)
        emit_pv(NJT - 2)
        emit_pv(NJT - 1)
        for u in fu:
            u()
        # numerators -> ah (f32), denominators -> dram (f16)
        def finish():
            dt = dent.tile([1, 2, 512], F16, name="den", tag="den")
            for h in range(2):
                hs = slice(h * 64, h * 64 + 64)
                nc.vector.tensor_copy(out=_r(ah[hs, qs]),
                                      in_=_r(pot[h][0:64, :]))
                nc.vector.tensor_copy(out=dt[:, h, :], in_=pot[h][64:65, :])
            nc.gpsimd.dma_start(out=dens2[qc:qc + 1, :, :], in_=dt[:])
        return finish

    # ---- prologue: x DMAs || q proj || P'(0) || k/v proj ----
    # P'(0) needs only qT[:, 0:512] (q-pair 0), so its serial chain through
    # the pp slot starts immediately; proj units keep the PE busy between
    # the P' casts.
    p0 = p_units_for(0, act_casts=True)  # 3 ci + 1 write per qt -> 16
    p0i = iter(p0)
    q_pair_unit(0)
    q_pair_unit(1)
    next(p0i)()                     # qt0 ci0
    k_pair_unit(0)
    next(p0i)(); next(p0i)()        # qt0 ci1, ci2
    k_pair_unit(1)
    next(p0i)(); next(p0i)()        # qt0 write, qt1 ci0
    v_group_unit(0)
    next(p0i)(); next(p0i)()        # qt1 ci1, ci2
    v_group_unit(1)
    next(p0i)(); next(p0i)()        # qt1 write, qt2 ci0
    v_group_unit(2)
    next(p0i)(); next(p0i)()        # qt2 ci1, ci2
    v_group_unit(3)
    for u in p0i:                   # qt2 write, qt3 all
        u()
    skt = emit_skew_read(0)

    # ---- main loop over q-chunks in order 0, 3, 1, 2 ----
    # Each chunk's skew DMA is produced during an earlier iteration.  All
    # P'-production units live in ONE global queue drained at a steady,
    # DVE-sustainable pace (~1.5 units/step) across iterations, so the DVE
    # cast chain never bunches up in a single iteration.  outproj units for
    # the previous chunk are prepended (PE+ACT only, no DVE).
    QSEQ = [0, 3, 1, 2]
    skew_done = {}
    global_q = []
    for i, nqc in enumerate(QSEQ[1:]):
        global_q += p_units_for(nqc)
        global_q.append(lambda nqc=nqc: skew_done.update(
            {nqc: emit_skew_read(nqc)}))
    gqi = [0]

    def drain_gq(n):
        while n > 0 and gqi[0] < len(global_q):
            global_q[gqi[0]]()
            gqi[0] += 1
            n -= 1

    finish_prev = None
    for i, qc in enumerate(QSEQ):
        pqc = QSEQ[i - 1] if i > 0 else None
        # the next chunk's skew must be issued before its attn begins;
        # normally the pace suffices, this is a safety net
        if qc not in skew_done and i > 0:
            while qc not in skew_done and gqi[0] < len(global_q):
                drain_gq(1)
        o_fills = ([finish_prev] if finish_prev else []) + o_units_for(pqc)
        finish_prev = emit_attn(qc, skt if i == 0 else skew_done[qc],
                                o_fills, drain_gq)

    # tail: numerator copies + outproj for the last q-chunk
    finish_prev()
    for u in o_units_for(QSEQ[-1]):
        u()
    ctx.close()


_NC_CACHE = [None]


def _get_nc():
    if _NC_CACHE[0] is None:
        _NC_CACHE[0] = build_kernel()
    return _NC_CACHE[0]


def make_in_maps(x, Wq, Wkv, Wo, bo, rel_emb):
    bf16 = ml_dtypes.bfloat16
    xT = [np.ascontiguousarray(x[b].T).astype(bf16) for b in range(2)]
    cols = np.arange(W)
    idx = np.clip(1535 - cols, 0, 1024)
    relT = np.empty((128, W), np.float32)
    relT[0:64] = rel_emb[idx].T
    relT[64:128] = relT[0:64]
    relT = relT.astype(bf16)                       # reversed rel table
    relbc = np.empty((128, 256), np.float32)
    relbc[0:64, 0:128] = rel_emb[0][:, None]       # clamp-low value
    relbc[0:64, 128:256] = rel_emb[1024][:, None]  # clamp-high value
    relbc[64:128] = relbc[0:64]
    relbc = relbc.astype(bf16)
    ident = np.eye(128, dtype=np.float16)
    in_maps = []
    for c in range(8):
        b, hp = c // 4, c % 4
        cs = slice(hp * 128, hp * 128 + 128)
        wqkv = np.empty((4, 128, 3, 128), np.float32)
        for ci in range(4):
            rows = slice(ci * 128, ci * 128 + 128)
            wqkv[ci, :, 0, :] = Wq[rows, cs] / 8.0
            wqkv[ci, :, 1, :] = Wkv[rows, :512][:, cs]
            wqkv[ci, :, 2, :] = Wkv[rows, 512:][:, cs]
        in_maps.append({
            "xT": xT[b],
            "wqkv": wqkv.astype(bf16),
            "wo2": np.ascontiguousarray(Wo[cs, :]).astype(np.float32),
            "relT": relT,
            "relbc": relbc,
            "ident": ident,
        })
    return in_maps


def run(x, Wq, Wkv, Wo, bo, rel_emb, trace=False, trace_cores=None):
    nc = _get_nc()
    in_maps = make_in_maps(x, Wq, Wkv, Wo, bo, rel_emb)
    res = run_bass_kernel_spmd(nc, in_maps, core_ids=list(range(8)),
                               trace=trace, trace_cores=trace_cores)
    out = np.zeros((2, N, D), np.float32)
    for c in range(8):
        b = c // 4
        num = np.asarray(res.results[c]["outh"], np.float32)   # [2, N, D]
        den = np.asarray(res.results[c]["dens2"], np.float32)  # [4, 2, 512]
        for h in range(2):
            out[b] += num[h] / den[:, h, :].reshape(N)[:, None]
    out += np.asarray(bo, np.float32)[None, None, :]
    return out, res


def kernel(x, Wq, Wkv, Wo, bo, rel_emb):
    out, _ = run(np.asarray(x), np.asarray(Wq), np.asarray(Wkv),
                 np.asarray(Wo), np.asarray(bo), np.asarray(rel_emb))
    return out
